# revision 13
# baseline (speedup 1.0000x reference)
"""Bass/Trainium2 kernel for nn_Attn_13846974562399.

Computes, for the reference module:
    proj   = enc @ W^T + bias          # [S, B, H]
    scores = einsum('bh,sbh->bs', hidden[0], proj)
    attn   = softmax(scores, axis=1)   # -> [B, 1, S]

Algebraic restructure:
    scores[b, s] = q[b] . enc[s, b] + (hidden[0,b] . bias),  q = hidden[0] @ W.
The per-b constant is invariant under softmax over s and is dropped.  q
([B, H], ~128 KB) is computed on the host in float64; the memory-bound work
(streaming the 268 MB encoder tensor + batched dot products) runs on 8
NeuronCores, data-parallel over batch (4 local batches per core).

Per-core device program (DMA-bound; ~358 GB/s/core HBM roofline ~87 us for
the 33 MB shard).  All HBM loads go through the sync-engine HWDGE ring,
which drains FIFO, so ring order is chosen explicitly:

  [q 12KB][qpe 0.5MB] then per t-pair: 3x 1MB DVE chunks, with the 8
  transposed PE chunks front-loaded two per t-pair over the first 4 pairs.

- Local batches 0..2 (DVE path): host layout [tp, b, p, 2h] with
  s = p*16 + 2*tp + t2 -- every (tp, b) unit is contiguous 1 MB.  48 fused
  scalar_tensor_tensor ops ((enc*1)*q with accum_out=sum_h) write
  scores[p, b, t].  (TENSOR_TENSOR_REDUCE crashes this runtime's NX ucode;
  scalar_tensor_tensor is the same fused ALU path.)  Per-b softmax:
  per-partition max (DVE) -> cross-partition max (GPSIMD all-reduce) ->
  exp with per-partition bias + fused free-dim sum (ACT) -> cross-partition
  sum (GPSIMD) -> reciprocal+scale (DVE) -> 8 KB DMA out.
- Local batch 3 (PE path): host transposes this batch to [hc, h, s]
  (8 chunks of 1 MB) and replicates its q to [hc, h, 128].  fp32 matmuls
  (exact: measured 1.8e-5 abs err on std-32 scores) accumulate
  scores_ps[m, s] = sum_h q[h]*enc_t[h, s] into PSUM [128, 2048], the same
  full score row replicated across all 128 partitions -- so softmax needs
  no cross-partition step: reduce_max over free (DVE) -> exp + fused sum
  (ACT) -> reciprocal (DVE) -> scale row 0 (ACT) -> 8 KB DMA out.  All of
  it is hidden mid-stream because the PE chunks are front-loaded.
- q for batches 0..2 is broadcast on-device: 12 KB DMA -> PE outer-product
  with a ones row -> PSUM -> per-b ACT copies to SBUF (keeps the replica
  off the HBM stream; gpsimd's custom-op library load is ~9 us into the
  kernel, too late for this).
"""

import numpy as np

import concourse.bacc as bacc
import concourse.bass as bass
import concourse.mybir as mybir
import concourse.tile as tile
from concourse.bass_isa import ReduceOp
from concourse.bass_utils import run_bass_kernel_spmd

S, B, H = 2048, 32, 1024
NCORES = 8
BL = B // NCORES          # 4 local batches per core
BD = BL - 1               # batches on the DVE path (0..2)
P = 128                   # SBUF partitions
NT = S // P               # 16 s-tiles; s = p*NT + t
NTP = NT // 2             # 8 t-pairs (1 MB DVE chunks)
HC = H // P               # 8 h-chunks for the PE path
MMN = 512                 # fp32 moving-operand max free dim
F32 = mybir.dt.float32

ENC_BUFS = 10             # in-flight 1 MB DVE chunks
PE_BUFS = 3               # in-flight 1 MB PE chunks

LAST_RESULTS = None
TRACE = False

_NC = None


def _build_bass():
    nc = bacc.Bacc()
    encd = nc.dram_tensor("encd", [NTP, BD, P, 2 * H], F32, kind="ExternalInput")
    encp = nc.dram_tensor("encp", [HC, P, S], F32, kind="ExternalInput")
    q = nc.dram_tensor("q", [1, BD * H], F32, kind="ExternalInput")
    qpe = nc.dram_tensor("qpe", [HC, P, P], F32, kind="ExternalInput")
    out3 = nc.dram_tensor("attn3", [P, BD, NT], F32, kind="ExternalOutput")
    outp = nc.dram_tensor("attnp", [1, S], F32, kind="ExternalOutput")

    mult = mybir.AluOpType.mult

    with tile.TileContext(nc) as tc:
        with (
            tc.tile_pool(name="encdp", bufs=ENC_BUFS) as encd_pool,
            tc.tile_pool(name="encpp", bufs=PE_BUFS) as encp_pool,
            tc.tile_pool(name="small", bufs=1) as small,
            tc.tile_pool(name="psum", bufs=1, space="PSUM") as psum_pool,
        ):
            # ---- FIFO ring head: q (12 KB) then qpe (0.5 MB). ----
            q0 = small.tile([1, BD * H], F32)
            nc.sync.dma_start(out=q0, in_=q.ap())
            qpe_sb = small.tile([P, HC, P], F32)
            nc.sync.dma_start(
                out=qpe_sb, in_=qpe.ap().rearrange("c h m -> h c m")
            )

            # On-device q broadcast for the DVE batches: ones[1,P].T @ q0.
            ones = small.tile([1, P], F32)
            nc.vector.memset(ones, 1.0)
            qps = psum_pool.tile([P, BD * H], F32, tag="ps")
            for k in range(BD * H // MMN):
                nc.tensor.matmul(
                    qps[:, k * MMN : (k + 1) * MMN],
                    ones[:],
                    q0[:, k * MMN : (k + 1) * MMN],
                    start=True,
                    stop=True,
                )
            qb = small.tile([P, BD, H], F32)
            for b in range(BD):
                nc.scalar.copy(out=qb[:, b, :], in_=qps[:, b * H : (b + 1) * H])

            scores = small.tile([P, BD, NT], F32)
            dummy = small.tile([P, 1], F32)
            m = small.tile([P, BD], F32)
            negm = small.tile([P, BD], F32)
            e = small.tile([P, BD, NT], F32)
            ssum = small.tile([P, BD], F32)
            rz = small.tile([P, BD], F32)
            attn_sb = small.tile([P, BD, NT], F32)

            # PE-path score accumulator: full score row replicated over all
            # 128 partitions.  Shares the PSUM slot with qps (tag="ps");
            # lifetimes are disjoint (first PE chunk arrives after the qb
            # copies drained qps).
            scores_ps = psum_pool.tile([P, S], F32, tag="ps")

            encd_ap = encd.ap()
            encp_ap = encp.ap()

            def stt(et_slice, b, t):
                nc.vector.scalar_tensor_tensor(
                    out=dummy.broadcast_to((P, H)),
                    in0=et_slice,
                    scalar=1.0,
                    in1=qb[:, b, :],
                    op0=mult,
                    op1=mult,
                    accum_out=scores[:, b, t : t + 1],
                )

            def pe_chunk(j):
                # scores_ps[:, k-range] += qpe[j].T @ encp[j][:, k-range]
                pc = encp_pool.tile([P, S], F32)
                nc.sync.dma_start(out=pc, in_=encp_ap[j])
                for k in range(S // MMN):
                    nc.tensor.matmul(
                        scores_ps[:, k * MMN : (k + 1) * MMN],
                        qpe_sb[:, j, :],
                        pc[:, k * MMN : (k + 1) * MMN],
                        start=(j == 0),
                        stop=(j == HC - 1),
                    )

            for tp in range(NTP):
                for b in range(BD):
                    et = encd_pool.tile([P, 2, H], F32)
                    nc.sync.dma_start(out=et, in_=encd_ap[tp, b])
                    for t2 in range(2):
                        stt(et[:, t2, :], b, 2 * tp + t2)
                        t = 2 * tp + t2
                        if t == NT - 1:
                            nc.vector.tensor_reduce(
                                out=m[:, b : b + 1],
                                in_=scores[:, b, :],
                                axis=mybir.AxisListType.X,
                                op=mybir.AluOpType.max,
                            )
                            nc.gpsimd.partition_all_reduce(
                                m[:, b : b + 1], m[:, b : b + 1], P, ReduceOp.max
                            )
                # Front-load the PE chunks: two per t-pair over the first
                # four pairs, so batch 3's scores (and its softmax) finish
                # mid-stream.
                if tp < 4:
                    pe_chunk(2 * tp)
                    pe_chunk(2 * tp + 1)
                if tp == 4:
                    # Batch-3 softmax, fully overlapped with the remaining
                    # DVE stream.  Every partition holds the whole score
                    # row, so no cross-partition reduction is needed.
                    m3 = small.tile([P, 1], F32)
                    nc.vector.tensor_reduce(
                        out=m3,
                        in_=scores_ps,
                        axis=mybir.AxisListType.X,
                        op=mybir.AluOpType.max,
                    )
                    negm3 = small.tile([P, 1], F32)
                    nc.vector.tensor_scalar_mul(out=negm3, in0=m3, scalar1=-1.0)
                    e3 = small.tile([P, S], F32)
                    ssum3 = small.tile([P, 1], F32)
                    nc.scalar.activation(
                        out=e3,
                        in_=scores_ps,
                        func=mybir.ActivationFunctionType.Exp,
                        bias=negm3,
                        scale=1.0,
                        accum_out=ssum3,
                    )
                    rz3 = small.tile([P, 1], F32)
                    nc.vector.reciprocal(rz3, ssum3)
                    attn3_sb = small.tile([1, S], F32)
                    nc.scalar.mul(out=attn3_sb, in_=e3[0:1, :], mul=rz3[0:1])
                    nc.sync.dma_start(out=outp.ap(), in_=attn3_sb)

            # DVE-path softmax epilogues.
            for b in range(BD):
                nc.vector.tensor_scalar_mul(
                    out=negm[:, b : b + 1], in0=m[:, b : b + 1], scalar1=-1.0
                )
                nc.scalar.activation(
                    out=e[:, b, :],
                    in_=scores[:, b, :],
                    func=mybir.ActivationFunctionType.Exp,
                    bias=negm[:, b : b + 1],
                    scale=1.0,
                    accum_out=ssum[:, b : b + 1],
                )
                nc.gpsimd.partition_all_reduce(
                    ssum[:, b : b + 1], ssum[:, b : b + 1], P, ReduceOp.add
                )
            for b in range(BD):
                nc.vector.reciprocal(rz[:, b : b + 1], ssum[:, b : b + 1])
                nc.vector.tensor_scalar_mul(
                    out=attn_sb[:, b, :], in0=e[:, b, :], scalar1=rz[:, b : b + 1]
                )
                nc.sync.dma_start(out=out3.ap()[:, b, :], in_=attn_sb[:, b, :])

    nc.compile()
    return nc


def kernel(hidden, encoder_outputs, W, b):
    global _NC, LAST_RESULTS
    hidden = np.asarray(hidden, dtype=np.float32)
    enc = np.asarray(encoder_outputs, dtype=np.float32)
    W = np.asarray(W, dtype=np.float32)

    # q = hidden[0] @ W (fp64 accumulate on host).  The bias adds a per-b
    # constant to the scores, which softmax cancels, so `b` is unused.
    q_full = (hidden[0].astype(np.float64) @ W.astype(np.float64)).astype(np.float32)

    in_maps = []
    for c in range(NCORES):
        b0 = BL * c
        enc_c3 = enc[:, b0 : b0 + BD, :]                    # [S, BD, H]
        # [tp, b, p, (t2 h)] with s = p*16 + 2*tp + t2: contiguous 1 MB units.
        encd = np.ascontiguousarray(
            enc_c3.reshape(P, NTP, 2, BD, H)
            .transpose(1, 3, 0, 2, 4)
            .reshape(NTP, BD, P, 2 * H)
        )
        # PE batch: [hc, h, s], contiguous 1 MB per hc.
        encp = np.ascontiguousarray(
            enc[:, b0 + BD, :].T.reshape(HC, P, S)
        )
        q_c = np.ascontiguousarray(q_full[b0 : b0 + BD].reshape(1, BD * H))
        qpe = np.ascontiguousarray(
            np.broadcast_to(
                q_full[b0 + BD].reshape(HC, P)[:, :, None], (HC, P, P)
            )
        )
        in_maps.append({"encd": encd, "encp": encp, "q": q_c, "qpe": qpe})

    if _NC is None:
        _NC = _build_bass()

    LAST_RESULTS = run_bass_kernel_spmd(
        _NC, in_maps, core_ids=list(range(NCORES)), trace=TRACE
    )

    out = np.empty((B, 1, S), dtype=np.float32)
    for c in range(NCORES):
        a3 = LAST_RESULTS.results[c]["attn3"]               # [P, BD, NT]
        out[BL * c : BL * c + BD, 0, :] = a3.transpose(1, 0, 2).reshape(BD, S)
        out[BL * c + BD, 0, :] = LAST_RESULTS.results[c]["attnp"][0]
    return out


# revision 17
# speedup vs baseline: 1.1646x; 1.1646x over previous
"""Bass/Trainium2 kernel for nn_Attn_13846974562399.

Computes, for the reference module:
    proj   = enc @ W^T + bias          # [S, B, H]
    scores = einsum('bh,sbh->bs', hidden[0], proj)
    attn   = softmax(scores, axis=1)   # -> [B, 1, S]

Algebraic restructure:
    scores[b, s] = q[b] . enc[s, b] + (hidden[0,b] . bias),  q = hidden[0] @ W.
The per-b constant is invariant under softmax over s and is dropped.  q
([B, H], ~128 KB) is computed on the host in float64; the memory-bound work
(streaming the 268 MB encoder tensor + batched dot products) runs on 8
NeuronCores, data-parallel over batch (4 local batches per core).

Per-core device program (DMA-bound; ~358 GB/s/core HBM roofline ~87 us for
the 33 MB shard).  All HBM loads go through the sync-engine HWDGE ring,
which drains FIFO, so ring order is chosen explicitly:

  [q 12KB][qpe 0.5MB] then per t-pair: 3x 1MB DVE chunks, with the 8
  transposed PE chunks front-loaded two per t-pair over the first 4 pairs.

- Local batches 0..2 (DVE path): host layout [tp, b, p, 2h] with
  s = p*16 + 2*tp + t2 -- every (tp, b) unit is contiguous 1 MB.  48 fused
  scalar_tensor_tensor ops ((enc*1)*q with accum_out=sum_h) write
  scores[p, b, t].  (TENSOR_TENSOR_REDUCE crashes this runtime's NX ucode;
  scalar_tensor_tensor is the same fused ALU path.)  Per-b softmax:
  per-partition max (DVE) -> cross-partition max (GPSIMD all-reduce) ->
  exp with per-partition bias + fused free-dim sum (ACT) -> cross-partition
  sum (GPSIMD) -> reciprocal+scale (DVE) -> 8 KB DMA out.
- Local batch 3 (PE path): host transposes this batch to [hc, h, s]
  (8 chunks of 1 MB) and replicates its q to [hc, h, 128].  fp32 matmuls
  (exact: measured 1.8e-5 abs err on std-32 scores) accumulate
  scores_ps[m, s] = sum_h q[h]*enc_t[h, s] into PSUM [128, 2048], the same
  full score row replicated across all 128 partitions -- so softmax needs
  no cross-partition step: reduce_max over free (DVE) -> exp + fused sum
  (ACT) -> reciprocal (DVE) -> scale row 0 (ACT) -> 8 KB DMA out.  All of
  it is hidden mid-stream because the PE chunks are front-loaded.
- q for batches 0..2 is broadcast on-device: 12 KB DMA -> PE outer-product
  with a ones row -> PSUM -> per-b ACT copies to SBUF (keeps the replica
  off the HBM stream; gpsimd's custom-op library load is ~9 us into the
  kernel, too late for this).
"""

import numpy as np

import concourse.bacc as bacc
import concourse.bass as bass
import concourse.mybir as mybir
import concourse.tile as tile
from concourse.bass_isa import ReduceOp
from concourse.bass_utils import run_bass_kernel_spmd

S, B, H = 2048, 32, 1024
NCORES = 8
BL = B // NCORES          # 4 local batches per core
BD = BL - 1               # batches on the DVE path (0..2)
P = 128                   # SBUF partitions
NT = S // P               # 16 s-tiles; s = p*NT + t
NTP = NT // 2             # 8 t-pairs (1 MB DVE chunks)
HC = H // P               # 8 h-chunks for the PE path
MMN = 512                 # fp32 moving-operand max free dim
F32 = mybir.dt.float32

ENC_BUFS = 10             # in-flight 1 MB DVE chunks
PE_BUFS = 6               # in-flight 1 MB PE chunks

LAST_RESULTS = None
TRACE = False

_NC = None


def _build_bass():
    nc = bacc.Bacc()
    encd = nc.dram_tensor("encd", [NTP, BD, P, 2 * H], F32, kind="ExternalInput")
    encp = nc.dram_tensor("encp", [HC, P, S], F32, kind="ExternalInput")
    q = nc.dram_tensor("q", [1, BD * H], F32, kind="ExternalInput")
    qpe = nc.dram_tensor("qpe", [HC, P, P], F32, kind="ExternalInput")
    out3 = nc.dram_tensor("attn3", [P, BD, NT], F32, kind="ExternalOutput")
    outp = nc.dram_tensor("attnp", [1, S], F32, kind="ExternalOutput")

    mult = mybir.AluOpType.mult

    with tile.TileContext(nc) as tc:
        with (
            tc.tile_pool(name="encdp", bufs=ENC_BUFS) as encd_pool,
            tc.tile_pool(name="encpp", bufs=PE_BUFS) as encp_pool,
            tc.tile_pool(name="small", bufs=1) as small,
            tc.tile_pool(name="psum", bufs=1, space="PSUM") as psum_pool,
        ):
            # ---- FIFO ring head: q (12 KB) only; qpe rides after the
            # first DVE triple (it is needed just before the first PE chunk).
            q0 = small.tile([1, BD * H], F32)
            nc.sync.dma_start(out=q0, in_=q.ap())

            # On-device q broadcast for the DVE batches: ones[1,P].T @ q0.
            # One PSUM tile per b so each qb copy waits only on its own two
            # matmuls.
            ones = small.tile([1, P], F32)
            nc.vector.memset(ones, 1.0)
            qb = small.tile([P, BD, H], F32)
            for b in range(BD):
                qps = psum_pool.tile([P, H], F32, tag=f"qps{b}")
                for k in range(H // MMN):
                    nc.tensor.matmul(
                        qps[:, k * MMN : (k + 1) * MMN],
                        ones[:],
                        q0[:, b * H + k * MMN : b * H + (k + 1) * MMN],
                        start=True,
                        stop=True,
                    )
                nc.scalar.copy(out=qb[:, b, :], in_=qps[:])

            scores = small.tile([P, BD, NT], F32)
            dummy = small.tile([P, 1], F32)
            m = small.tile([P, BD], F32)
            negm = small.tile([P, BD], F32)
            e = small.tile([P, BD, NT], F32)
            ssum = small.tile([P, BD], F32)
            rz = small.tile([P, BD], F32)
            attn_sb = small.tile([P, BD, NT], F32)

            # PE-path score accumulator: full score row replicated over all
            # 128 partitions.  Shares qps0's PSUM slot (sized to the max of
            # the tag, 8 KB/partition; 8 banks total across the three tags);
            # lifetimes are disjoint (the first PE chunk arrives after the
            # qb copies drained the qps tiles).
            scores_ps = psum_pool.tile([P, S], F32, tag="qps0")
            qpe_sb = small.tile([P, HC, P], F32)

            encd_ap = encd.ap()
            encp_ap = encp.ap()

            def stt(et_slice, b, t):
                nc.vector.scalar_tensor_tensor(
                    out=dummy.broadcast_to((P, H)),
                    in0=et_slice,
                    scalar=1.0,
                    in1=qb[:, b, :],
                    op0=mult,
                    op1=mult,
                    accum_out=scores[:, b, t : t + 1],
                )

            def pe_chunk(j):
                # scores_ps[:, k-range] += qpe[j].T @ encp[j][:, k-range]
                pc = encp_pool.tile([P, S], F32)
                nc.sync.dma_start(out=pc, in_=encp_ap[j])
                for k in range(S // MMN):
                    nc.tensor.matmul(
                        scores_ps[:, k * MMN : (k + 1) * MMN],
                        qpe_sb[:, j, :],
                        pc[:, k * MMN : (k + 1) * MMN],
                        start=(j == 0),
                        stop=(j == HC - 1),
                    )

            for tp in range(NTP):
                for b in range(BD):
                    et = encd_pool.tile([P, 2, H], F32)
                    nc.sync.dma_start(out=et, in_=encd_ap[tp, b])
                    for t2 in range(2):
                        stt(et[:, t2, :], b, 2 * tp + t2)
                        t = 2 * tp + t2
                        if t == NT - 1:
                            nc.vector.tensor_reduce(
                                out=m[:, b : b + 1],
                                in_=scores[:, b, :],
                                axis=mybir.AxisListType.X,
                                op=mybir.AluOpType.max,
                            )
                            nc.gpsimd.partition_all_reduce(
                                m[:, b : b + 1], m[:, b : b + 1], P, ReduceOp.max
                            )
                # Front-load the PE chunks: two per t-pair over the first
                # four pairs, so batch 3's scores (and its softmax) finish
                # mid-stream.  qpe rides the ring after the first DVE triple.
                if tp == 0:
                    nc.sync.dma_start(
                        out=qpe_sb, in_=qpe.ap().rearrange("c h m -> h c m")
                    )
                if tp < 4:
                    pe_chunk(2 * tp)
                    pe_chunk(2 * tp + 1)
                if tp == 4:
                    # Batch-3 softmax, fully overlapped with the remaining
                    # DVE stream.  Every partition holds the whole score
                    # row, so no cross-partition reduction is needed.
                    m3 = small.tile([P, 1], F32)
                    nc.vector.tensor_reduce(
                        out=m3,
                        in_=scores_ps,
                        axis=mybir.AxisListType.X,
                        op=mybir.AluOpType.max,
                    )
                    negm3 = small.tile([P, 1], F32)
                    nc.vector.tensor_scalar_mul(out=negm3, in0=m3, scalar1=-1.0)
                    e3 = small.tile([P, S], F32)
                    ssum3 = small.tile([P, 1], F32)
                    nc.scalar.activation(
                        out=e3,
                        in_=scores_ps,
                        func=mybir.ActivationFunctionType.Exp,
                        bias=negm3,
                        scale=1.0,
                        accum_out=ssum3,
                    )
                    rz3 = small.tile([P, 1], F32)
                    nc.vector.reciprocal(rz3, ssum3)
                    attn3_sb = small.tile([1, S], F32)
                    nc.scalar.mul(out=attn3_sb, in_=e3[0:1, :], mul=rz3[0:1])
                    nc.sync.dma_start(out=outp.ap(), in_=attn3_sb)

            # DVE-path softmax epilogues.
            for b in range(BD):
                nc.vector.tensor_scalar_mul(
                    out=negm[:, b : b + 1], in0=m[:, b : b + 1], scalar1=-1.0
                )
                nc.scalar.activation(
                    out=e[:, b, :],
                    in_=scores[:, b, :],
                    func=mybir.ActivationFunctionType.Exp,
                    bias=negm[:, b : b + 1],
                    scale=1.0,
                    accum_out=ssum[:, b : b + 1],
                )
                nc.gpsimd.partition_all_reduce(
                    ssum[:, b : b + 1], ssum[:, b : b + 1], P, ReduceOp.add
                )
            for b in range(BD):
                nc.vector.reciprocal(rz[:, b : b + 1], ssum[:, b : b + 1])
                nc.vector.tensor_scalar_mul(
                    out=attn_sb[:, b, :], in0=e[:, b, :], scalar1=rz[:, b : b + 1]
                )
                nc.sync.dma_start(out=out3.ap()[:, b, :], in_=attn_sb[:, b, :])

    nc.compile()
    return nc


def kernel(hidden, encoder_outputs, W, b):
    global _NC, LAST_RESULTS
    hidden = np.asarray(hidden, dtype=np.float32)
    enc = np.asarray(encoder_outputs, dtype=np.float32)
    W = np.asarray(W, dtype=np.float32)

    # q = hidden[0] @ W (fp64 accumulate on host).  The bias adds a per-b
    # constant to the scores, which softmax cancels, so `b` is unused.
    q_full = (hidden[0].astype(np.float64) @ W.astype(np.float64)).astype(np.float32)

    in_maps = []
    for c in range(NCORES):
        b0 = BL * c
        enc_c3 = enc[:, b0 : b0 + BD, :]                    # [S, BD, H]
        # [tp, b, p, (t2 h)] with s = p*16 + 2*tp + t2: contiguous 1 MB units.
        encd = np.ascontiguousarray(
            enc_c3.reshape(P, NTP, 2, BD, H)
            .transpose(1, 3, 0, 2, 4)
            .reshape(NTP, BD, P, 2 * H)
        )
        # PE batch: [hc, h, s], contiguous 1 MB per hc.
        encp = np.ascontiguousarray(
            enc[:, b0 + BD, :].T.reshape(HC, P, S)
        )
        q_c = np.ascontiguousarray(q_full[b0 : b0 + BD].reshape(1, BD * H))
        qpe = np.ascontiguousarray(
            np.broadcast_to(
                q_full[b0 + BD].reshape(HC, P)[:, :, None], (HC, P, P)
            )
        )
        in_maps.append({"encd": encd, "encp": encp, "q": q_c, "qpe": qpe})

    if _NC is None:
        _NC = _build_bass()

    LAST_RESULTS = run_bass_kernel_spmd(
        _NC, in_maps, core_ids=list(range(NCORES)), trace=TRACE
    )

    out = np.empty((B, 1, S), dtype=np.float32)
    for c in range(NCORES):
        a3 = LAST_RESULTS.results[c]["attn3"]               # [P, BD, NT]
        out[BL * c : BL * c + BD, 0, :] = a3.transpose(1, 0, 2).reshape(BD, S)
        out[BL * c + BD, 0, :] = LAST_RESULTS.results[c]["attnp"][0]
    return out
